# revision 33
# baseline (speedup 1.0000x reference)
"""Trainium2 Bass kernel for ConnectedFilterLayerWithImplicitJacobian.

Merged-stream formulation: the host (index marshalling only) interleaves each
core's 1/8 slice of the 2N Euler-tour delta slots with its sorted-by-t pixels
into one stream; pixel rows are zero rows.  On-chip, per core:

  A'. filtered = sigmoid(attrs @ w + b) * (sign * residues) streamed over the
      merged slice (zero rows stay exactly 0.0).  The K=8 GEMV runs as fused
      scalar_tensor_tensor (attrs_k * w_k + acc) passes over bf16 k-plane
      chunks on two HWDGE queues; the filtered mul carries accum_out row
      totals for the cross-partition fix.
  S.  base_r = sum of filtered over the DFS stack at the slice start
      (nodes with tpre < r*TS <= tpost; <= tree depth ~40 rows)
  D'. native tensor_tensor_scan prefix sum (fp32 state, bf16 samples),
      seeded with initial = triangular-PE row-prefix + base_r, in two
      chained segments so the first half's DMA overlaps the second scan.

Every pixel's output y[p] = cumsum value at its merged position - no gather.
Host extracts pixel positions and unpermutes (index marshalling).
"""
import contextlib
import ctypes
import os
import sys
import types

sys.path.insert(0, "/opt/trn_rl_repo")

import numpy as np

# ---------------------------------------------------------------- shims ----
_SO_PATH = "/opt/axon/libaxon_pjrt.so"


def _install_ntff_shim():
    if "antenv.axon_hooks" in sys.modules:
        return
    try:
        lib = ctypes.CDLL(_SO_PATH)
        ok = hasattr(lib, "axon_start_nrt_profile")
    except OSError:
        ok = False
    if ok:
        lib.axon_start_nrt_profile.argtypes = [ctypes.POINTER(ctypes.c_int64), ctypes.c_size_t]
        lib.axon_start_nrt_profile.restype = ctypes.c_int64
        lib.axon_stop_nrt_profile.argtypes = [ctypes.c_char_p]
        lib.axon_stop_nrt_profile.restype = ctypes.c_int64

        @contextlib.contextmanager
        def _hook(output_dir, device_ids):
            import jax

            jax.devices()
            if device_ids:
                ids = (ctypes.c_int64 * len(device_ids))(*device_ids)
                rc = lib.axon_start_nrt_profile(ids, len(device_ids))
            else:
                rc = lib.axon_start_nrt_profile(None, 0)
            if rc != 0:
                raise RuntimeError(f"axon_start_nrt_profile rc={rc}")
            try:
                yield
            finally:
                n = lib.axon_stop_nrt_profile(str(output_dir).encode())
                if n < 0:
                    raise RuntimeError(f"axon_stop_nrt_profile rc={n}")
    else:
        _hook = None
    mod = types.ModuleType("antenv.axon_hooks")
    mod.get_axon_ntff_profile_hook = lambda: _hook
    mod.set_axon_ntff_profile_hook = lambda h: None
    sys.modules["antenv.axon_hooks"] = mod


_install_ntff_shim()

import concourse.bass as bass
import concourse.bass_utils as bass_utils
import concourse.mybir as mybir
import concourse.tile as tile
from concourse.bass_utils import run_bass_kernel_spmd

# walrus birsim on a large program is prohibitively slow; turn it off
_orig_run_command = bass_utils.run_command


def _patched_run_command(argv, **kwargs):
    argv = ["--enable-birsim=false" if a == "--enable-birsim=true" else a for a in argv]
    return _orig_run_command(argv, **kwargs)


bass_utils.run_command = _patched_run_command

MAX_WAITS = 1


def _split_excess_waits(nc):
    """This container's walrus accepts at most one sync-wait per instruction;
    move extra waits onto injected no-ops ahead of the instruction."""
    nid = 0
    for bb in nc.main_func.blocks:
        insts = bb.instructions
        targets = []
        for idx in range(len(insts)):
            ins = insts[idx]
            si = ins.sync_info
            if si is not None and si.on_wait is not None and len(si.on_wait) > MAX_WAITS:
                targets.append(ins.name)
        for name in targets:
            idx = next(i for i in range(len(insts)) if insts[i].name == name)
            ins = insts[idx]
            w = list(ins.sync_info.on_wait)
            excess, keep = w[:-MAX_WAITS], w[-MAX_WAITS:]
            ins.sync_info.on_wait = keep
            pos = idx
            while excess:
                chunk, excess = excess[:MAX_WAITS], excess[MAX_WAITS:]
                nop = mybir.InstNoOp(
                    name=f"I-ws-{nid}", engine=ins.engine, ins=[], outs=[],
                    sync_info=mybir.SyncInfo(on_wait=chunk, on_update=[]),
                )
                nid += 1
                insts.insert(pos, nop)
                pos += 1


# ------------------------------------------------------------- geometry ----
NCORES = 8
N = 500_000
K = 8
ROWS, COLS = 2048, 4096
P = ROWS * COLS                  # 8388608
T2N = 2 * N                      # 1000000
DSZ = 128 * 7816                 # 1000448 padded tour length
TS = DSZ // NCORES               # 125056 tour slots per core slice
FDV = 9232                       # merged columns per partition
CAP = 128 * FDV                  # 1181696 merged capacity per core
CHUNKS = [577, 577] + [1154] * 7         # phase-A' chunk widths (sum = FDV)
NCH = len(CHUNKS)
CH = 1154                        # legacy host-layout chunk width

_cache = {}
_last_res = [None]


def _build_program():
    if "nc" in _cache:
        return _cache["nc"]
    nc = bass.Bass()
    f32 = mybir.dt.float32
    bf16 = mybir.dt.bfloat16

    w_sc = nc.dram_tensor("w_sc", [128, K], f32, kind="ExternalInput")
    b_rep = nc.dram_tensor("b_rep", [128, 1], f32, kind="ExternalInput")
    ltm = nc.dram_tensor("ltm", [128, 128], f32, kind="ExternalInput")
    attrs_m = nc.dram_tensor("attrs_m", [128, FDV * K], bf16, kind="ExternalInput")
    res_m = nc.dram_tensor("res_m", [128, FDV], f32, kind="ExternalInput")
    sa = nc.dram_tensor("sa", [128, K], f32, kind="ExternalInput")
    sr = nc.dram_tensor("sr", [128, 1], f32, kind="ExternalInput")
    y_mrg = nc.dram_tensor("y_mrg", [128, FDV], bf16, kind="ExternalOutput")

    mult, add = mybir.AluOpType.mult, mybir.AluOpType.add

    with tile.TileContext(nc) as tc:
        with tc.tile_pool(name="keep", bufs=1) as kp:
            lt_t = kp.tile([128, 128], f32)
            one_t = kp.tile([128, 128], f32)
            b_t = kp.tile([128, 1], f32)
            sb_t = kp.tile([128, 1], f32)
            sk_a = kp.tile([128, K], f32)
            sk_r = kp.tile([128, 1], f32)
            w_t = kp.tile([128, K], f32)
            nc.gpsimd.dma_start(lt_t[:], ltm[:])
            nc.scalar.dma_start(b_t[:], b_rep[:])
            nc.gpsimd.dma_start(sk_a[:], sa[:])
            nc.gpsimd.dma_start(sk_r[:], sr[:])
            nc.scalar.dma_start(w_t[:], w_sc[:])
            nc.vector.memset(one_t[:], 1.0)

            with tc.tile_pool(name="scan", bufs=1) as sp:
                wa = sp.tile([128, FDV], f32)
                wb = sp.tile([128, FDV], f32)
                off_sb = sp.tile([128, 1], f32)

                # ---- phase A': filtered over the merged stream ----
                # attrs chunk layout is k-plane-major: at[:, k*ch:(k+1)*ch]
                # holds plane k; logits built by fused (attrs_k * w_k) + acc.
                bypass = mybir.AluOpType.bypass
                rowts = []
                with tc.tile_pool(name="pa", bufs=4) as pa, \
                     tc.tile_pool(name="rp", bufs=NCH) as rp:
                    c0 = 0
                    for c, ch in enumerate(CHUNKS):
                        at = pa.tile([128, ch * K], bf16)
                        rt = pa.tile([128, ch], f32)
                        l0 = pa.tile([128, ch], f32)
                        l1 = pa.tile([128, ch], f32)
                        rowt = rp.tile([128, 1], f32)
                        half = ch * K // 2
                        rh = ch // 2
                        qa = nc.scalar if c % 2 == 0 else nc.sync
                        qb = nc.sync if c % 2 == 0 else nc.scalar
                        qa.dma_start(at[:, :half],
                                     attrs_m[:, c0 * K:c0 * K + half])
                        qb.dma_start(
                            at[:, half:], attrs_m[:, c0 * K + half:(c0 + ch) * K])
                        nc.gpsimd.dma_start(rt[:], res_m[:, c0:c0 + ch])
                        acc, nac = l0, l1
                        nc.vector.tensor_scalar_mul(
                            acc[:], at[:, 0:ch], w_t[:, 0:1])
                        for k in range(1, K):
                            nc.vector.scalar_tensor_tensor(
                                nac[:], at[:, k * ch:(k + 1) * ch],
                                w_t[:, k:k + 1], acc[:], mult, add)
                            acc, nac = nac, acc
                        nc.scalar.activation(
                            acc[:], acc[:],
                            mybir.ActivationFunctionType.Sigmoid,
                            bias=b_t[:], scale=1.0,
                        )
                        nc.vector.scalar_tensor_tensor(
                            wa[:, c0:c0 + ch], acc[:], 1.0, rt[:],
                            mult, mult, accum_out=rowt[:])
                        rowts.append(rowt)
                        c0 += ch

                    # ---- phase S: base from the DFS stack rows ----
                    s0 = pa.tile([128, 1], f32)
                    s1 = pa.tile([128, 1], f32)
                    acc, nac = s0, s1
                    nc.vector.tensor_scalar_mul(
                        acc[:], sk_a[:, 0:1], w_t[:, 0:1])
                    for k in range(1, K):
                        nc.vector.scalar_tensor_tensor(
                            nac[:], sk_a[:, k:k + 1], w_t[:, k:k + 1],
                            acc[:], mult, add)
                        acc, nac = nac, acc
                    nc.scalar.activation(
                        acc[:], acc[:], mybir.ActivationFunctionType.Sigmoid,
                        bias=b_t[:], scale=1.0,
                    )
                    nc.vector.tensor_mul(sb_t[:], acc[:], sk_r[:])

                    # ---- row totals -> cross-partition fix (before scan) ----
                    rtot = sp.tile([128, 1], f32)
                    nc.vector.tensor_add(rtot[:], rowts[0][:], rowts[1][:])
                    for rowt in rowts[2:]:
                        nc.vector.tensor_add(rtot[:], rtot[:], rowt[:])
                    with tc.tile_pool(name="psc", bufs=1, space="PSUM") as pp:
                        ps = pp.tile([128, 1], f32, space="PSUM")
                        nc.tensor.matmul(
                            ps[:], lhsT=lt_t[:], rhs=rtot[:],
                            start=True, stop=False,
                        )
                        nc.tensor.matmul(
                            ps[:], lhsT=one_t[:], rhs=sb_t[:],
                            start=False, stop=True,
                        )
                        nc.vector.tensor_copy(off_sb[:], ps[:])

                # ---- phase D': seeded prefix scan, segmented for overlap ----
                # scan state is fp32 internally; stored samples downcast to
                # bf16 (independent rounding, no error accumulation)
                wo = wb[:].bitcast(bf16)[:, :FDV]
                SEGQ = [(0, 4616, nc.sync), (4616, 7500, nc.scalar),
                        (7500, FDV, nc.sync)]
                for i, (s0, s1, q) in enumerate(SEGQ):
                    init = off_sb[:, 0:1] if i == 0 else wo[:, s0 - 1:s0]
                    nc.vector.tensor_tensor_scan(
                        wo[:, s0:s1], wa[:, s0:s1], wa[:, s0:s1], init,
                        add, bypass)
                    q.dma_start(y_mrg[:, s0:s1], wo[:, s0:s1])

    _split_excess_waits(nc)
    _cache["nc"] = nc
    return nc


def kernel(weight, bias, residues, attrs2d, tpre, tpost, node_of_pixel,
           numRows, numCols, _profile=[None]):
    weight = np.asarray(weight, np.float32)
    bias = np.asarray(bias, np.float32)
    residues = np.asarray(residues, np.float32)
    attrs2d = np.asarray(attrs2d, np.float32)
    tpre = np.asarray(tpre, np.int64)
    tpost = np.asarray(tpost, np.int64)
    nop = np.asarray(node_of_pixel, np.int64)
    numRows = int(numRows)
    numCols = int(numCols)

    # --- host-side marshalling (indices only; the one float op is the exact
    # sign flip res * (+-1)) ---
    ordr = np.zeros(DSZ, np.int64)
    sgn = np.zeros(DSZ, np.float32)
    ar = np.arange(N)
    ordr[tpre] = ar
    sgn[tpre] = 1.0
    ordr[tpost] = ar
    sgn[tpost] = -1.0
    res_signed = residues[ordr] * sgn          # (DSZ,) exact +-residue, 0 on pads

    t_pix = tpre[nop]                          # (P,) pixel tour positions
    order = np.argsort(t_pix, kind="stable")
    t_sorted = t_pix[order]
    slice_of = t_sorted // TS
    counts = np.bincount(slice_of, minlength=NCORES)
    starts = np.concatenate(([0], np.cumsum(counts))).astype(np.int64)

    w_sc = np.tile(weight[None, :], (128, 1)).astype(np.float32)
    b_rep = np.full((128, 1), np.float32(bias[0]), np.float32)
    lt = (np.arange(128)[:, None] < np.arange(128)[None, :]).astype(np.float32)

    in_maps = []
    pix_pos = []
    for r in range(NCORES):
        t0 = r * TS
        ts_r = t_sorted[starts[r]:starts[r + 1]] - t0       # pixel tau (local)
        mpos_pix = ts_r + 1 + np.arange(len(ts_r))
        t_slice = np.arange(TS)
        cnt_lt = np.searchsorted(ts_r, t_slice, side="left")
        mpos_del = t_slice + cnt_lt
        assert len(ts_r) + TS <= CAP, (r, len(ts_r))

        attrs_mr = np.zeros((CAP, K), np.float32)
        res_mr = np.zeros(CAP, np.float32)
        gsl = slice(t0, t0 + TS)
        attrs_mr[mpos_del] = attrs2d[ordr[gsl]]
        attrs_mr[mpos_del[sgn[gsl] == 0.0]] = 0.0
        res_mr[mpos_del] = res_signed[gsl]

        smask = (tpre < t0) & (tpost >= t0)
        sn = np.where(smask)[0]
        assert len(sn) <= 128, len(sn)
        sa_r = np.zeros((128, K), np.float32)
        sr_r = np.zeros((128, 1), np.float32)
        sa_r[:len(sn)] = attrs2d[sn]
        sr_r[:len(sn), 0] = residues[sn]

        # chunk-local k-plane-major layout, variable chunk widths
        import ml_dtypes
        a3 = attrs_mr.reshape(128, FDV, K)
        blocks = []
        c0 = 0
        for ch in CHUNKS:
            blocks.append(a3[:, c0:c0 + ch, :].transpose(0, 2, 1)
                          .reshape(128, ch * K))
            c0 += ch
        attrs_kp = np.concatenate(blocks, axis=1).astype(ml_dtypes.bfloat16)
        in_maps.append({
            "w_sc": w_sc,
            "b_rep": b_rep,
            "ltm": lt,
            "attrs_m": np.ascontiguousarray(attrs_kp),
            "res_m": res_mr.reshape(128, FDV),
            "sa": sa_r,
            "sr": sr_r,
        })
        pix_pos.append(mpos_pix)

    nc = _build_program()
    res = run_bass_kernel_spmd(nc, in_maps, list(range(NCORES)),
                               trace=bool(_profile[0]))
    _last_res[0] = res
    if _profile[0] is not None:
        _profile[0] = res.exec_time_ns

    y_sorted = np.concatenate([
        res.results[r]["y_mrg"].reshape(-1)[pix_pos[r]].astype(np.float32)
        for r in range(NCORES)
    ])
    y = np.empty(P, np.float32)
    y[order] = y_sorted
    return y.reshape(numRows, numCols)


# revision 34
# speedup vs baseline: 1.0278x; 1.0278x over previous
"""Trainium2 Bass kernel for ConnectedFilterLayerWithImplicitJacobian.

Merged-stream formulation: the host (index marshalling only) interleaves each
core's 1/8 slice of the 2N Euler-tour delta slots with its sorted-by-t pixels
into one stream; pixel rows are zero rows.  On-chip, per core:

  A'. filtered = sigmoid(attrs @ w + b) * (sign * residues) streamed over the
      merged slice (zero rows stay exactly 0.0).  The K=8 GEMV runs as fused
      scalar_tensor_tensor (attrs_k * w_k + acc) passes over bf16 k-plane
      chunks on two HWDGE queues; the filtered mul carries accum_out row
      totals for the cross-partition fix.
  S.  base_r = sum of filtered over the DFS stack at the slice start
      (nodes with tpre < r*TS <= tpost; <= tree depth ~40 rows)
  D'. native tensor_tensor_scan prefix sum (fp32 state, bf16 samples),
      seeded with initial = triangular-PE row-prefix + base_r, in two
      chained segments so the first half's DMA overlaps the second scan.

Every pixel's output y[p] = cumsum value at its merged position - no gather.
Host extracts pixel positions and unpermutes (index marshalling).
"""
import contextlib
import ctypes
import os
import sys
import types

sys.path.insert(0, "/opt/trn_rl_repo")

import numpy as np

# ---------------------------------------------------------------- shims ----
_SO_PATH = "/opt/axon/libaxon_pjrt.so"


def _install_ntff_shim():
    if "antenv.axon_hooks" in sys.modules:
        return
    try:
        lib = ctypes.CDLL(_SO_PATH)
        ok = hasattr(lib, "axon_start_nrt_profile")
    except OSError:
        ok = False
    if ok:
        lib.axon_start_nrt_profile.argtypes = [ctypes.POINTER(ctypes.c_int64), ctypes.c_size_t]
        lib.axon_start_nrt_profile.restype = ctypes.c_int64
        lib.axon_stop_nrt_profile.argtypes = [ctypes.c_char_p]
        lib.axon_stop_nrt_profile.restype = ctypes.c_int64

        @contextlib.contextmanager
        def _hook(output_dir, device_ids):
            import jax

            jax.devices()
            if device_ids:
                ids = (ctypes.c_int64 * len(device_ids))(*device_ids)
                rc = lib.axon_start_nrt_profile(ids, len(device_ids))
            else:
                rc = lib.axon_start_nrt_profile(None, 0)
            if rc != 0:
                raise RuntimeError(f"axon_start_nrt_profile rc={rc}")
            try:
                yield
            finally:
                n = lib.axon_stop_nrt_profile(str(output_dir).encode())
                if n < 0:
                    raise RuntimeError(f"axon_stop_nrt_profile rc={n}")
    else:
        _hook = None
    mod = types.ModuleType("antenv.axon_hooks")
    mod.get_axon_ntff_profile_hook = lambda: _hook
    mod.set_axon_ntff_profile_hook = lambda h: None
    sys.modules["antenv.axon_hooks"] = mod


_install_ntff_shim()

import concourse.bass as bass
import concourse.bass_utils as bass_utils
import concourse.mybir as mybir
import concourse.tile as tile
from concourse.bass_utils import run_bass_kernel_spmd

# walrus birsim on a large program is prohibitively slow; turn it off
_orig_run_command = bass_utils.run_command


def _patched_run_command(argv, **kwargs):
    argv = ["--enable-birsim=false" if a == "--enable-birsim=true" else a for a in argv]
    return _orig_run_command(argv, **kwargs)


bass_utils.run_command = _patched_run_command

MAX_WAITS = 1


def _split_excess_waits(nc):
    """This container's walrus accepts at most one sync-wait per instruction;
    move extra waits onto injected no-ops ahead of the instruction."""
    nid = 0
    for bb in nc.main_func.blocks:
        insts = bb.instructions
        targets = []
        for idx in range(len(insts)):
            ins = insts[idx]
            si = ins.sync_info
            if si is not None and si.on_wait is not None and len(si.on_wait) > MAX_WAITS:
                targets.append(ins.name)
        for name in targets:
            idx = next(i for i in range(len(insts)) if insts[i].name == name)
            ins = insts[idx]
            w = list(ins.sync_info.on_wait)
            excess, keep = w[:-MAX_WAITS], w[-MAX_WAITS:]
            ins.sync_info.on_wait = keep
            pos = idx
            while excess:
                chunk, excess = excess[:MAX_WAITS], excess[MAX_WAITS:]
                nop = mybir.InstNoOp(
                    name=f"I-ws-{nid}", engine=ins.engine, ins=[], outs=[],
                    sync_info=mybir.SyncInfo(on_wait=chunk, on_update=[]),
                )
                nid += 1
                insts.insert(pos, nop)
                pos += 1


# ------------------------------------------------------------- geometry ----
NCORES = 8
N = 500_000
K = 8
ROWS, COLS = 2048, 4096
P = ROWS * COLS                  # 8388608
T2N = 2 * N                      # 1000000
DSZ = 128 * 7816                 # 1000448 padded tour length
TS = DSZ // NCORES               # 125056 tour slots per core slice
FDV = 9232                       # merged columns per partition
CAP = 128 * FDV                  # 1181696 merged capacity per core
CHUNKS = [577, 577] + [1154] * 7         # phase-A' chunk widths (sum = FDV)
NCH = len(CHUNKS)
CH = 1154                        # legacy host-layout chunk width

_cache = {}
_last_res = [None]


def _build_program():
    if "nc" in _cache:
        return _cache["nc"]
    nc = bass.Bass()
    f32 = mybir.dt.float32
    bf16 = mybir.dt.bfloat16

    w_sc = nc.dram_tensor("w_sc", [128, K], f32, kind="ExternalInput")
    b_rep = nc.dram_tensor("b_rep", [128, 1], f32, kind="ExternalInput")
    ltm = nc.dram_tensor("ltm", [128, 128], f32, kind="ExternalInput")
    attrs_m = nc.dram_tensor("attrs_m", [128, FDV * K], bf16, kind="ExternalInput")
    res_m = nc.dram_tensor("res_m", [128, FDV], f32, kind="ExternalInput")
    sa = nc.dram_tensor("sa", [128, K], f32, kind="ExternalInput")
    sr = nc.dram_tensor("sr", [128, 1], f32, kind="ExternalInput")
    y_mrg = nc.dram_tensor("y_mrg", [128, FDV], bf16, kind="ExternalOutput")

    mult, add = mybir.AluOpType.mult, mybir.AluOpType.add

    with tile.TileContext(nc) as tc:
        with tc.tile_pool(name="keep", bufs=1) as kp:
            lt_t = kp.tile([128, 128], f32)
            one_t = kp.tile([128, 128], f32)
            b_t = kp.tile([128, 1], f32)
            sb_t = kp.tile([128, 1], f32)
            sk_a = kp.tile([128, K], f32)
            sk_r = kp.tile([128, 1], f32)
            w_t = kp.tile([128, K], f32)
            nc.gpsimd.dma_start(lt_t[:], ltm[:])
            nc.scalar.dma_start(b_t[:], b_rep[:])
            nc.gpsimd.dma_start(sk_a[:], sa[:])
            nc.gpsimd.dma_start(sk_r[:], sr[:])
            nc.scalar.dma_start(w_t[:], w_sc[:])
            nc.vector.memset(one_t[:], 1.0)

            with tc.tile_pool(name="scan", bufs=1) as sp:
                wa = sp.tile([128, FDV], f32)
                wb = sp.tile([128, FDV], f32)
                off_sb = sp.tile([128, 1], f32)

                # ---- phase A': filtered over the merged stream ----
                # attrs chunk layout is k-plane-major: at[:, k*ch:(k+1)*ch]
                # holds plane k; logits built by fused (attrs_k * w_k) + acc.
                bypass = mybir.AluOpType.bypass
                rowts = []
                with tc.tile_pool(name="pa", bufs=4) as pa, \
                     tc.tile_pool(name="rp", bufs=NCH) as rp:
                    c0 = 0
                    for c, ch in enumerate(CHUNKS):
                        at = pa.tile([128, ch * K], bf16)
                        rt = pa.tile([128, ch], f32)
                        l0 = pa.tile([128, ch], f32)
                        l1 = pa.tile([128, ch], f32)
                        rowt = rp.tile([128, 1], f32)
                        half = ch * K // 2
                        rh = ch // 2
                        qa = nc.scalar if c % 2 == 0 else nc.sync
                        qb = nc.sync if c % 2 == 0 else nc.scalar
                        qa.dma_start(at[:, :half],
                                     attrs_m[:, c0 * K:c0 * K + half])
                        qb.dma_start(
                            at[:, half:], attrs_m[:, c0 * K + half:(c0 + ch) * K])
                        qb.dma_start(rt[:, :rh], res_m[:, c0:c0 + rh])
                        qa.dma_start(rt[:, rh:], res_m[:, c0 + rh:c0 + ch])
                        acc, nac = l0, l1
                        nc.vector.tensor_scalar_mul(
                            acc[:], at[:, 0:ch], w_t[:, 0:1])
                        for k in range(1, K):
                            nc.vector.scalar_tensor_tensor(
                                nac[:], at[:, k * ch:(k + 1) * ch],
                                w_t[:, k:k + 1], acc[:], mult, add)
                            acc, nac = nac, acc
                        nc.scalar.activation(
                            acc[:], acc[:],
                            mybir.ActivationFunctionType.Sigmoid,
                            bias=b_t[:], scale=1.0,
                        )
                        nc.vector.scalar_tensor_tensor(
                            wa[:, c0:c0 + ch], acc[:], 1.0, rt[:],
                            mult, mult, accum_out=rowt[:])
                        rowts.append(rowt)
                        c0 += ch

                    # ---- phase S: base from the DFS stack rows ----
                    s0 = pa.tile([128, 1], f32)
                    s1 = pa.tile([128, 1], f32)
                    acc, nac = s0, s1
                    nc.vector.tensor_scalar_mul(
                        acc[:], sk_a[:, 0:1], w_t[:, 0:1])
                    for k in range(1, K):
                        nc.vector.scalar_tensor_tensor(
                            nac[:], sk_a[:, k:k + 1], w_t[:, k:k + 1],
                            acc[:], mult, add)
                        acc, nac = nac, acc
                    nc.scalar.activation(
                        acc[:], acc[:], mybir.ActivationFunctionType.Sigmoid,
                        bias=b_t[:], scale=1.0,
                    )
                    nc.vector.tensor_mul(sb_t[:], acc[:], sk_r[:])

                    # ---- row totals -> cross-partition fix (before scan) ----
                    rtot = sp.tile([128, 1], f32)
                    nc.vector.tensor_add(rtot[:], rowts[0][:], rowts[1][:])
                    for rowt in rowts[2:]:
                        nc.vector.tensor_add(rtot[:], rtot[:], rowt[:])
                    with tc.tile_pool(name="psc", bufs=1, space="PSUM") as pp:
                        ps = pp.tile([128, 1], f32, space="PSUM")
                        nc.tensor.matmul(
                            ps[:], lhsT=lt_t[:], rhs=rtot[:],
                            start=True, stop=False,
                        )
                        nc.tensor.matmul(
                            ps[:], lhsT=one_t[:], rhs=sb_t[:],
                            start=False, stop=True,
                        )
                        nc.vector.tensor_copy(off_sb[:], ps[:])

                # ---- phase D': seeded prefix scan, segmented for overlap ----
                # scan state is fp32 internally; stored samples downcast to
                # bf16 (independent rounding, no error accumulation)
                wo = wb[:].bitcast(bf16)[:, :FDV]
                SEGQ = [(0, 4616, nc.sync), (4616, 7500, nc.scalar),
                        (7500, FDV, nc.sync)]
                for i, (s0, s1, q) in enumerate(SEGQ):
                    init = off_sb[:, 0:1] if i == 0 else wo[:, s0 - 1:s0]
                    nc.vector.tensor_tensor_scan(
                        wo[:, s0:s1], wa[:, s0:s1], wa[:, s0:s1], init,
                        add, bypass)
                    q.dma_start(y_mrg[:, s0:s1], wo[:, s0:s1])

    _split_excess_waits(nc)
    _cache["nc"] = nc
    return nc


def kernel(weight, bias, residues, attrs2d, tpre, tpost, node_of_pixel,
           numRows, numCols, _profile=[None]):
    weight = np.asarray(weight, np.float32)
    bias = np.asarray(bias, np.float32)
    residues = np.asarray(residues, np.float32)
    attrs2d = np.asarray(attrs2d, np.float32)
    tpre = np.asarray(tpre, np.int64)
    tpost = np.asarray(tpost, np.int64)
    nop = np.asarray(node_of_pixel, np.int64)
    numRows = int(numRows)
    numCols = int(numCols)

    # --- host-side marshalling (indices only; the one float op is the exact
    # sign flip res * (+-1)) ---
    ordr = np.zeros(DSZ, np.int64)
    sgn = np.zeros(DSZ, np.float32)
    ar = np.arange(N)
    ordr[tpre] = ar
    sgn[tpre] = 1.0
    ordr[tpost] = ar
    sgn[tpost] = -1.0
    res_signed = residues[ordr] * sgn          # (DSZ,) exact +-residue, 0 on pads

    t_pix = tpre[nop]                          # (P,) pixel tour positions
    order = np.argsort(t_pix, kind="stable")
    t_sorted = t_pix[order]
    slice_of = t_sorted // TS
    counts = np.bincount(slice_of, minlength=NCORES)
    starts = np.concatenate(([0], np.cumsum(counts))).astype(np.int64)

    w_sc = np.tile(weight[None, :], (128, 1)).astype(np.float32)
    b_rep = np.full((128, 1), np.float32(bias[0]), np.float32)
    lt = (np.arange(128)[:, None] < np.arange(128)[None, :]).astype(np.float32)

    in_maps = []
    pix_pos = []
    for r in range(NCORES):
        t0 = r * TS
        ts_r = t_sorted[starts[r]:starts[r + 1]] - t0       # pixel tau (local)
        mpos_pix = ts_r + 1 + np.arange(len(ts_r))
        t_slice = np.arange(TS)
        cnt_lt = np.searchsorted(ts_r, t_slice, side="left")
        mpos_del = t_slice + cnt_lt
        assert len(ts_r) + TS <= CAP, (r, len(ts_r))

        attrs_mr = np.zeros((CAP, K), np.float32)
        res_mr = np.zeros(CAP, np.float32)
        gsl = slice(t0, t0 + TS)
        attrs_mr[mpos_del] = attrs2d[ordr[gsl]]
        attrs_mr[mpos_del[sgn[gsl] == 0.0]] = 0.0
        res_mr[mpos_del] = res_signed[gsl]

        smask = (tpre < t0) & (tpost >= t0)
        sn = np.where(smask)[0]
        assert len(sn) <= 128, len(sn)
        sa_r = np.zeros((128, K), np.float32)
        sr_r = np.zeros((128, 1), np.float32)
        sa_r[:len(sn)] = attrs2d[sn]
        sr_r[:len(sn), 0] = residues[sn]

        # chunk-local k-plane-major layout, variable chunk widths
        import ml_dtypes
        a3 = attrs_mr.reshape(128, FDV, K)
        blocks = []
        c0 = 0
        for ch in CHUNKS:
            blocks.append(a3[:, c0:c0 + ch, :].transpose(0, 2, 1)
                          .reshape(128, ch * K))
            c0 += ch
        attrs_kp = np.concatenate(blocks, axis=1).astype(ml_dtypes.bfloat16)
        in_maps.append({
            "w_sc": w_sc,
            "b_rep": b_rep,
            "ltm": lt,
            "attrs_m": np.ascontiguousarray(attrs_kp),
            "res_m": res_mr.reshape(128, FDV),
            "sa": sa_r,
            "sr": sr_r,
        })
        pix_pos.append(mpos_pix)

    nc = _build_program()
    res = run_bass_kernel_spmd(nc, in_maps, list(range(NCORES)),
                               trace=bool(_profile[0]))
    _last_res[0] = res
    if _profile[0] is not None:
        _profile[0] = res.exec_time_ns

    y_sorted = np.concatenate([
        res.results[r]["y_mrg"].reshape(-1)[pix_pos[r]].astype(np.float32)
        for r in range(NCORES)
    ])
    y = np.empty(P, np.float32)
    y[order] = y_sorted
    return y.reshape(numRows, numCols)


# revision 35
# speedup vs baseline: 1.1904x; 1.1582x over previous
"""Trainium2 Bass kernel for ConnectedFilterLayerWithImplicitJacobian.

Merged-stream formulation: the host (index marshalling only) interleaves each
core's 1/8 slice of the 2N Euler-tour delta slots with its sorted-by-t pixels
into one stream; pixel rows are zero rows.  On-chip, per core:

  A'. filtered = sigmoid(attrs @ w + b) * (sign * residues) streamed over the
      merged slice (zero rows stay exactly 0.0).  The K=8 GEMV runs as fused
      scalar_tensor_tensor (attrs_k * w_k + acc) passes over bf16 k-plane
      chunks on two HWDGE queues; the filtered mul carries accum_out row
      totals for the cross-partition fix.
  S.  base_r = sum of filtered over the DFS stack at the slice start
      (nodes with tpre < r*TS <= tpost; <= tree depth ~40 rows)
  D'. native tensor_tensor_scan prefix sum (fp32 state, bf16 samples),
      seeded with initial = triangular-PE row-prefix + base_r, in two
      chained segments so the first half's DMA overlaps the second scan.

Every pixel's output y[p] = cumsum value at its merged position - no gather.
Host extracts pixel positions and unpermutes (index marshalling).
"""
import contextlib
import ctypes
import os
import sys
import types

sys.path.insert(0, "/opt/trn_rl_repo")

import numpy as np

# ---------------------------------------------------------------- shims ----
_SO_PATH = "/opt/axon/libaxon_pjrt.so"


def _install_ntff_shim():
    if "antenv.axon_hooks" in sys.modules:
        return
    try:
        lib = ctypes.CDLL(_SO_PATH)
        ok = hasattr(lib, "axon_start_nrt_profile")
    except OSError:
        ok = False
    if ok:
        lib.axon_start_nrt_profile.argtypes = [ctypes.POINTER(ctypes.c_int64), ctypes.c_size_t]
        lib.axon_start_nrt_profile.restype = ctypes.c_int64
        lib.axon_stop_nrt_profile.argtypes = [ctypes.c_char_p]
        lib.axon_stop_nrt_profile.restype = ctypes.c_int64

        @contextlib.contextmanager
        def _hook(output_dir, device_ids):
            import jax

            jax.devices()
            if device_ids:
                ids = (ctypes.c_int64 * len(device_ids))(*device_ids)
                rc = lib.axon_start_nrt_profile(ids, len(device_ids))
            else:
                rc = lib.axon_start_nrt_profile(None, 0)
            if rc != 0:
                raise RuntimeError(f"axon_start_nrt_profile rc={rc}")
            try:
                yield
            finally:
                n = lib.axon_stop_nrt_profile(str(output_dir).encode())
                if n < 0:
                    raise RuntimeError(f"axon_stop_nrt_profile rc={n}")
    else:
        _hook = None
    mod = types.ModuleType("antenv.axon_hooks")
    mod.get_axon_ntff_profile_hook = lambda: _hook
    mod.set_axon_ntff_profile_hook = lambda h: None
    sys.modules["antenv.axon_hooks"] = mod


_install_ntff_shim()

import concourse.bass as bass
import concourse.bass_utils as bass_utils
import concourse.mybir as mybir
import concourse.tile as tile
from concourse.bass_utils import run_bass_kernel_spmd

# walrus birsim on a large program is prohibitively slow; turn it off
_orig_run_command = bass_utils.run_command


def _patched_run_command(argv, **kwargs):
    argv = ["--enable-birsim=false" if a == "--enable-birsim=true" else a for a in argv]
    return _orig_run_command(argv, **kwargs)


bass_utils.run_command = _patched_run_command

MAX_WAITS = 1


def _split_excess_waits(nc):
    """This container's walrus accepts at most one sync-wait per instruction;
    move extra waits onto injected no-ops ahead of the instruction."""
    nid = 0
    for bb in nc.main_func.blocks:
        insts = bb.instructions
        targets = []
        for idx in range(len(insts)):
            ins = insts[idx]
            si = ins.sync_info
            if si is not None and si.on_wait is not None and len(si.on_wait) > MAX_WAITS:
                targets.append(ins.name)
        for name in targets:
            idx = next(i for i in range(len(insts)) if insts[i].name == name)
            ins = insts[idx]
            w = list(ins.sync_info.on_wait)
            excess, keep = w[:-MAX_WAITS], w[-MAX_WAITS:]
            ins.sync_info.on_wait = keep
            pos = idx
            while excess:
                chunk, excess = excess[:MAX_WAITS], excess[MAX_WAITS:]
                nop = mybir.InstNoOp(
                    name=f"I-ws-{nid}", engine=ins.engine, ins=[], outs=[],
                    sync_info=mybir.SyncInfo(on_wait=chunk, on_update=[]),
                )
                nid += 1
                insts.insert(pos, nop)
                pos += 1


# ------------------------------------------------------------- geometry ----
NCORES = 8
N = 500_000
K = 8
ROWS, COLS = 2048, 4096
P = ROWS * COLS                  # 8388608
T2N = 2 * N                      # 1000000
DSZ = 128 * 7816                 # 1000448 padded tour length
TS = DSZ // NCORES               # 125056 tour slots per core slice
FDV = 9232                       # merged columns per partition
CAP = 128 * FDV                  # 1181696 merged capacity per core
CHUNKS = [577, 577] + [1154] * 7         # phase-A' chunk widths (sum = FDV)
NCH = len(CHUNKS)
CH = 1154                        # legacy host-layout chunk width

_cache = {}
_last_res = [None]


def _build_program():
    if "nc" in _cache:
        return _cache["nc"]
    nc = bass.Bass()
    f32 = mybir.dt.float32
    bf16 = mybir.dt.bfloat16

    w_sc = nc.dram_tensor("w_sc", [128, K], f32, kind="ExternalInput")
    b_rep = nc.dram_tensor("b_rep", [128, 1], f32, kind="ExternalInput")
    ltm = nc.dram_tensor("ltm", [128, 128], f32, kind="ExternalInput")
    attrs_m = nc.dram_tensor("attrs_m", [128, FDV * K], bf16, kind="ExternalInput")
    res_m = nc.dram_tensor("res_m", [128, FDV], f32, kind="ExternalInput")
    sa = nc.dram_tensor("sa", [128, K], f32, kind="ExternalInput")
    sr = nc.dram_tensor("sr", [128, 1], f32, kind="ExternalInput")
    y_mrg = nc.dram_tensor("y_mrg", [128, FDV], bf16, kind="ExternalOutput")

    mult, add = mybir.AluOpType.mult, mybir.AluOpType.add

    with tile.TileContext(nc) as tc:
        with tc.tile_pool(name="keep", bufs=1) as kp:
            lt_t = kp.tile([128, 128], f32)
            one_t = kp.tile([128, 128], f32)
            b_t = kp.tile([128, 1], f32)
            sb_t = kp.tile([128, 1], f32)
            sk_a = kp.tile([128, K], f32)
            sk_r = kp.tile([128, 1], f32)
            w_t = kp.tile([128, K], f32)
            nc.gpsimd.dma_start(lt_t[:], ltm[:])
            nc.gpsimd.dma_start(b_t[:], b_rep[:])
            nc.gpsimd.dma_start(sk_a[:], sa[:])
            nc.gpsimd.dma_start(sk_r[:], sr[:])
            nc.gpsimd.dma_start(w_t[:], w_sc[:])
            nc.vector.memset(one_t[:], 1.0)

            with tc.tile_pool(name="scan", bufs=1) as sp:
                wa = sp.tile([128, FDV], f32)
                wb = sp.tile([128, FDV], f32)
                off_sb = sp.tile([128, 1], f32)

                # ---- phase A': filtered over the merged stream ----
                # attrs chunk layout is k-plane-major: at[:, k*ch:(k+1)*ch]
                # holds plane k; logits built by fused (attrs_k * w_k) + acc.
                bypass = mybir.AluOpType.bypass
                rowts = []
                with tc.tile_pool(name="pa", bufs=4) as pa, \
                     tc.tile_pool(name="rp", bufs=NCH) as rp:
                    c0 = 0
                    for c, ch in enumerate(CHUNKS):
                        at = pa.tile([128, ch * K], bf16)
                        rt = pa.tile([128, ch], f32)
                        l0 = pa.tile([128, ch], f32)
                        l1 = pa.tile([128, ch], f32)
                        rowt = rp.tile([128, 1], f32)
                        half = ch * K // 2
                        rh = ch // 2
                        qa = nc.scalar if c % 2 == 0 else nc.sync
                        qb = nc.sync if c % 2 == 0 else nc.scalar
                        qa.dma_start(at[:, :half],
                                     attrs_m[:, c0 * K:c0 * K + half])
                        qb.dma_start(
                            at[:, half:], attrs_m[:, c0 * K + half:(c0 + ch) * K])
                        qb.dma_start(rt[:, :rh], res_m[:, c0:c0 + rh])
                        qa.dma_start(rt[:, rh:], res_m[:, c0 + rh:c0 + ch])
                        acc, nac = l0, l1
                        nc.vector.tensor_scalar_mul(
                            acc[:], at[:, 0:ch], w_t[:, 0:1])
                        for k in range(1, K):
                            nc.vector.scalar_tensor_tensor(
                                nac[:], at[:, k * ch:(k + 1) * ch],
                                w_t[:, k:k + 1], acc[:], mult, add)
                            acc, nac = nac, acc
                        nc.scalar.activation(
                            acc[:], acc[:],
                            mybir.ActivationFunctionType.Sigmoid,
                            bias=b_t[:], scale=1.0,
                        )
                        nc.vector.scalar_tensor_tensor(
                            wa[:, c0:c0 + ch], acc[:], 1.0, rt[:],
                            mult, mult, accum_out=rowt[:])
                        rowts.append(rowt)
                        c0 += ch

                    # ---- phase S: base from the DFS stack rows ----
                    s0 = pa.tile([128, 1], f32)
                    s1 = pa.tile([128, 1], f32)
                    acc, nac = s0, s1
                    nc.vector.tensor_scalar_mul(
                        acc[:], sk_a[:, 0:1], w_t[:, 0:1])
                    for k in range(1, K):
                        nc.vector.scalar_tensor_tensor(
                            nac[:], sk_a[:, k:k + 1], w_t[:, k:k + 1],
                            acc[:], mult, add)
                        acc, nac = nac, acc
                    nc.scalar.activation(
                        acc[:], acc[:], mybir.ActivationFunctionType.Sigmoid,
                        bias=b_t[:], scale=1.0,
                    )
                    nc.vector.tensor_mul(sb_t[:], acc[:], sk_r[:])

                    # ---- row totals -> cross-partition fix (before scan) ----
                    rtot = sp.tile([128, 1], f32)
                    nc.vector.tensor_add(rtot[:], rowts[0][:], rowts[1][:])
                    for rowt in rowts[2:]:
                        nc.vector.tensor_add(rtot[:], rtot[:], rowt[:])
                    with tc.tile_pool(name="psc", bufs=1, space="PSUM") as pp:
                        ps = pp.tile([128, 1], f32, space="PSUM")
                        nc.tensor.matmul(
                            ps[:], lhsT=lt_t[:], rhs=rtot[:],
                            start=True, stop=False,
                        )
                        nc.tensor.matmul(
                            ps[:], lhsT=one_t[:], rhs=sb_t[:],
                            start=False, stop=True,
                        )
                        nc.vector.tensor_copy(off_sb[:], ps[:])

                # ---- phase D': seeded prefix scan, segmented for overlap ----
                # scan state is fp32 internally; stored samples downcast to
                # bf16 (independent rounding, no error accumulation)
                wo = wb[:].bitcast(bf16)[:, :FDV]
                SEGQ = [(0, 4616, nc.sync), (4616, 7500, nc.scalar),
                        (7500, FDV, nc.sync)]
                for i, (s0, s1, q) in enumerate(SEGQ):
                    init = off_sb[:, 0:1] if i == 0 else wo[:, s0 - 1:s0]
                    nc.vector.tensor_tensor_scan(
                        wo[:, s0:s1], wa[:, s0:s1], wa[:, s0:s1], init,
                        add, bypass)
                    q.dma_start(y_mrg[:, s0:s1], wo[:, s0:s1])

    _split_excess_waits(nc)
    _cache["nc"] = nc
    return nc


def kernel(weight, bias, residues, attrs2d, tpre, tpost, node_of_pixel,
           numRows, numCols, _profile=[None]):
    weight = np.asarray(weight, np.float32)
    bias = np.asarray(bias, np.float32)
    residues = np.asarray(residues, np.float32)
    attrs2d = np.asarray(attrs2d, np.float32)
    tpre = np.asarray(tpre, np.int64)
    tpost = np.asarray(tpost, np.int64)
    nop = np.asarray(node_of_pixel, np.int64)
    numRows = int(numRows)
    numCols = int(numCols)

    # --- host-side marshalling (indices only; the one float op is the exact
    # sign flip res * (+-1)) ---
    ordr = np.zeros(DSZ, np.int64)
    sgn = np.zeros(DSZ, np.float32)
    ar = np.arange(N)
    ordr[tpre] = ar
    sgn[tpre] = 1.0
    ordr[tpost] = ar
    sgn[tpost] = -1.0
    res_signed = residues[ordr] * sgn          # (DSZ,) exact +-residue, 0 on pads

    t_pix = tpre[nop]                          # (P,) pixel tour positions
    order = np.argsort(t_pix, kind="stable")
    t_sorted = t_pix[order]
    slice_of = t_sorted // TS
    counts = np.bincount(slice_of, minlength=NCORES)
    starts = np.concatenate(([0], np.cumsum(counts))).astype(np.int64)

    w_sc = np.tile(weight[None, :], (128, 1)).astype(np.float32)
    b_rep = np.full((128, 1), np.float32(bias[0]), np.float32)
    lt = (np.arange(128)[:, None] < np.arange(128)[None, :]).astype(np.float32)

    in_maps = []
    pix_pos = []
    for r in range(NCORES):
        t0 = r * TS
        ts_r = t_sorted[starts[r]:starts[r + 1]] - t0       # pixel tau (local)
        mpos_pix = ts_r + 1 + np.arange(len(ts_r))
        t_slice = np.arange(TS)
        cnt_lt = np.searchsorted(ts_r, t_slice, side="left")
        mpos_del = t_slice + cnt_lt
        assert len(ts_r) + TS <= CAP, (r, len(ts_r))

        attrs_mr = np.zeros((CAP, K), np.float32)
        res_mr = np.zeros(CAP, np.float32)
        gsl = slice(t0, t0 + TS)
        attrs_mr[mpos_del] = attrs2d[ordr[gsl]]
        attrs_mr[mpos_del[sgn[gsl] == 0.0]] = 0.0
        res_mr[mpos_del] = res_signed[gsl]

        smask = (tpre < t0) & (tpost >= t0)
        sn = np.where(smask)[0]
        assert len(sn) <= 128, len(sn)
        sa_r = np.zeros((128, K), np.float32)
        sr_r = np.zeros((128, 1), np.float32)
        sa_r[:len(sn)] = attrs2d[sn]
        sr_r[:len(sn), 0] = residues[sn]

        # chunk-local k-plane-major layout, variable chunk widths
        import ml_dtypes
        a3 = attrs_mr.reshape(128, FDV, K)
        blocks = []
        c0 = 0
        for ch in CHUNKS:
            blocks.append(a3[:, c0:c0 + ch, :].transpose(0, 2, 1)
                          .reshape(128, ch * K))
            c0 += ch
        attrs_kp = np.concatenate(blocks, axis=1).astype(ml_dtypes.bfloat16)
        in_maps.append({
            "w_sc": w_sc,
            "b_rep": b_rep,
            "ltm": lt,
            "attrs_m": np.ascontiguousarray(attrs_kp),
            "res_m": res_mr.reshape(128, FDV),
            "sa": sa_r,
            "sr": sr_r,
        })
        pix_pos.append(mpos_pix)

    nc = _build_program()
    res = run_bass_kernel_spmd(nc, in_maps, list(range(NCORES)),
                               trace=bool(_profile[0]))
    _last_res[0] = res
    if _profile[0] is not None:
        _profile[0] = res.exec_time_ns

    y_sorted = np.concatenate([
        res.results[r]["y_mrg"].reshape(-1)[pix_pos[r]].astype(np.float32)
        for r in range(NCORES)
    ])
    y = np.empty(P, np.float32)
    y[order] = y_sorted
    return y.reshape(numRows, numCols)
